# revision 1
# baseline (speedup 1.0000x reference)
"""Trainium2 Bass kernel for NeuralDecisionTree (soft decision tree MoE).

Strategy: data-parallel over batch across 8 NeuronCores (1024 rows/core),
weights replicated.  All matmuls run in float32r (reduced-precision fp32
multiply, fp32 accumulate) at full PE streaming rate.  fp32r matmuls must
write PSUM at partition base 0 with even stationary width, which shapes the
PSUM layout below.

Per-core dataflow (activations kept in [feature, batch] layout throughout):
  router:  z = router_W @ x^T          (4 K-chunk matmuls, M padded to 64)
  S      = [ln s; 0; ln(1-s); 0]       (128 rows; s = sigmoid(z + rb))
  log p  = A @ S  -> p = exp(A @ S)    per-leaf soft routing products in
           log space via 0/1 selection matmuls; pre-broadcast into the
           pred layout (8 output slots per leaf).
  L1:     h1T_l = relu(W1_l^T @ x^T + b1)      per leaf, 4 K-chunk matmuls
  L2:     h2T per leaf [64,N] PSUM -> bias+relu into a paired [128,N] SBUF
  L3:     pred for a leaf pair via block-diagonal W3 pack [128,32] ->
          [32,N] PSUM (16 real rows + 16 zero rows)
  mix:    prod[32jj:+32] = pred * p_arr[32jj:+32] (DVE into stacked SBUF),
          out_t += R^T @ prod (0/1 matmul), plus out_t += b3^T @ pT.
"""

import os
import sys

import numpy as np

if "/opt/trn_rl_repo" not in sys.path:
    sys.path.insert(0, "/opt/trn_rl_repo")

import concourse.bass as bass
import concourse.hw_specs as hw_specs
import concourse.tile as tile
from concourse import bacc, mybir
from concourse.bass_utils import run_bass_kernel_spmd

_ONE_TABLE = "natural_log_exp_and_others"
_orig_get_tables = hw_specs.get_activation_tables


def _patched_get_tables(module_arch):
    """Confine activation-table choice to one set that covers every ACT
    func this kernel uses (exp/ln/relu/abs/copy/identity), so the greedy
    per-instruction table picker never ping-pongs between sets.  Dict
    order (= act_func_set_id) is preserved; other sets are emptied."""
    tables = dict(_orig_get_tables(module_arch))
    keep = tables[_ONE_TABLE]
    return {k: (v if k == _ONE_TABLE else (v & set()) or set())
            if k != _ONE_TABLE else keep for k, v in tables.items()}

f32 = mybir.dt.float32
f32r = mybir.dt.float32r
AF = mybir.ActivationFunctionType
ALU = mybir.AluOpType

# Problem shape (hardcoded; harness contract)
B = 8192
D = 512
H1 = 128
H2 = 64
OUT = 8
L = 64
NI = 63
NCORES = 8
BC = B // NCORES        # 1024 rows per core
N = 512                 # batch tile (matmul free dim / PSUM bank)
T = BC // N             # 2 batch tiles per core
KC = D // 128           # 4 contraction chunks for the input dim
NPAIR = L // 2          # 32 leaf pairs
NG = 8                  # 8-leaf groups


def _leaf_path_rows(leaf):
    """Rows of the [128] log-sigmoid stack contributing to log p(leaf).

    Row n (n<63) holds ln d_n; row 64+n holds ln(1-d_n); rows 63 and 127
    are zero pads.  Mirrors the reference's level-wise p interleave.
    """
    rows = []
    for k in range(6):
        prefix = leaf >> (6 - k)
        node = (2 ** k - 1) + prefix
        bit = (leaf >> (5 - k)) & 1
        rows.append(node + 64 * bit)
    return rows


def build_nc():
    nc = bacc.Bacc("TRN2", target_bir_lowering=False, debug=False,
                   num_devices=NCORES)
    bacc_mod = sys.modules["concourse.bacc"]
    bacc_mod.get_activation_tables = _patched_get_tables

    d_xa = nc.dram_tensor("xa", [128, T, KC, N], f32r, kind="ExternalInput").ap()
    d_w1 = nc.dram_tensor("w1a", [128, L, KC, 128], f32r, kind="ExternalInput").ap()
    d_rw = nc.dram_tensor("rwa", [128, KC, 64], f32r, kind="ExternalInput").ap()
    d_w2 = nc.dram_tensor("w2a", [128, NPAIR, 2, H2], f32r, kind="ExternalInput").ap()
    d_w3 = nc.dram_tensor("w3p", [128, NPAIR, 32], f32r, kind="ExternalInput").ap()
    d_a64 = nc.dram_tensor("a64", [128, L], f32r, kind="ExternalInput").ap()
    d_afl = nc.dram_tensor("afull", [128, 1024], f32r, kind="ExternalInput").ap()
    d_r = nc.dram_tensor("rsel", [128, OUT], f32r, kind="ExternalInput").ap()
    d_b3 = nc.dram_tensor("b3t", [L, OUT], f32r, kind="ExternalInput").ap()
    d_b1 = nc.dram_tensor("b1a", [128, L], f32, kind="ExternalInput").ap()
    d_b2 = nc.dram_tensor("b2a", [128, NPAIR], f32, kind="ExternalInput").ap()
    d_rbp = nc.dram_tensor("rbp", [64, 1], f32, kind="ExternalInput").ap()
    d_rbn = nc.dram_tensor("rbn", [64, 1], f32, kind="ExternalInput").ap()
    d_out = nc.dram_tensor("outT", [OUT, BC], f32, kind="ExternalOutput").ap()

    with tile.TileContext(nc) as tc:
        with tc.tile_pool(name="const", bufs=1) as cpool, \
             tc.tile_pool(name="w1", bufs=2) as w1pool, \
             tc.tile_pool(name="w2w3", bufs=3) as h2wpool, \
             tc.tile_pool(name="spool", bufs=2) as spool, \
             tc.tile_pool(name="ptpool", bufs=2) as ptpool, \
             tc.tile_pool(name="parr", bufs=3) as papool, \
             tc.tile_pool(name="h1", bufs=4) as h1pool, \
             tc.tile_pool(name="h2", bufs=3) as h2pool, \
             tc.tile_pool(name="prod", bufs=3) as prpool, \
             tc.tile_pool(name="osb", bufs=2) as opool, \
             tc.tile_pool(name="ps_h1", bufs=3, space="PSUM") as ps_h1, \
             tc.tile_pool(name="ps_h2", bufs=3, space="PSUM") as ps_h2, \
             tc.tile_pool(name="ps_pm", bufs=2, space="PSUM") as ps_pm:

            # ---- constants into SBUF, in byte-arrival order ----
            # One serial sync queue controls exactly which bytes land first:
            # router weights (warmup/z) -> x -> g0's L2/L3 weights + A
            # matrices -> W1 pair chunks.  Tiny biases ride the scalar queue.
            rwa = cpool.tile([128, KC, 64], f32r)
            nc.sync.dma_start(rwa[:], d_rw)
            xa = cpool.tile([128, T, KC, N], f32r)
            for c in range(KC):
                nc.sync.dma_start(xa[:, 0, c, :], d_xa[:, 0, c, :])
            w1g0 = w1pool.tile([128, 8, KC, 128], f32r, tag="w1", name="w1g0")
            nc.sync.dma_start(w1g0[:, 0:2, :, :], d_w1[:, 0:2, :, :])
            w2g0 = h2wpool.tile([128, 4, 2, H2], f32r, tag="w2", name="w2g0")
            nc.sync.dma_start(w2g0[:], d_w2[:, 0:4, :, :])
            w3g0 = h2wpool.tile([128, 4, 32], f32r, tag="w3", name="w3g0")
            nc.sync.dma_start(w3g0[:], d_w3[:, 0:4, :])
            a64 = cpool.tile([128, L], f32r)
            nc.sync.dma_start(a64[:], d_a64)
            afl = cpool.tile([128, 1024], f32r)
            nc.sync.dma_start(afl[:, 0:256], d_afl[:, 0:256])
            for c in range(KC):
                nc.sync.dma_start(xa[:, 1, c, :], d_xa[:, 1, c, :])
            nc.sync.dma_start(w1g0[:, 2:4, :, :], d_w1[:, 2:4, :, :])
            nc.sync.dma_start(afl[:, 256:1024], d_afl[:, 256:1024])
            rsel = cpool.tile([128, OUT], f32r)
            nc.sync.dma_start(rsel[:], d_r)
            b3t = cpool.tile([L, OUT], f32r)
            nc.sync.dma_start(b3t[:], d_b3)
            nc.sync.dma_start(w1g0[:, 4:6, :, :], d_w1[:, 4:6, :, :])
            nc.sync.dma_start(w1g0[:, 6:8, :, :], d_w1[:, 6:8, :, :])
            rbp = cpool.tile([64, 1], f32)
            nc.scalar.dma_start(rbp[:], d_rbp)
            rbn = cpool.tile([64, 1], f32)
            nc.scalar.dma_start(rbn[:], d_rbn)
            b1a = cpool.tile([128, L], f32)
            nc.scalar.dma_start(b1a[:], d_b1)
            b2a = cpool.tile([128, NPAIR], f32)
            nc.scalar.dma_start(b2a[:], d_b2)

            # ---- routing + first L1 pair, interleaved for startup ----
            # PE order: z(t0) -> L1 pair0 t0 (needs only the first W1 chunk)
            # -> z(t1) (by which time the second half of x has landed).
            # Row 63 of z is a zero pad from the padded router weights.
            #   ln s     = -(relu(-z') + ln(1 + exp(-|z'|)))
            #   ln (1-s) = -(relu( z') + ln(1 + exp(-|z'|)))
            def emit_l1_leaf(s, t, jj, e, w1g):
                leaf = 2 * s + e
                h1_ps = ps_h1.tile([128, N], f32, tag="h1",
                                   name=f"h1ps{s}_{t}_{e}")
                for c in range(KC):
                    nc.tensor.matmul(
                        h1_ps[:], w1g[:, 2 * jj + e, c, :],
                        xa[:, t, c, :],
                        start=(c == 0), stop=(c == KC - 1))
                h1_t = h1pool.tile([128, N], f32r, tag="h1s",
                                   name=f"h1s{s}_{t}_{e}")
                nc.scalar.activation(h1_t[:], h1_ps[:], AF.Relu,
                                     bias=b1a[:, leaf:leaf + 1],
                                     scale=1.0)
                return h1_t

            def emit_l1(s, t, jj, w1g):
                return [emit_l1_leaf(s, t, jj, e, w1g) for e in range(2)]

            z_pss = []
            pre_h1 = {}
            for t in range(T):
                z_ps = ps_pm.tile([64, N], f32, tag="pm", name=f"z_ps{t}")
                for c in range(KC):
                    nc.tensor.matmul(z_ps[:], rwa[:, c, :],
                                     xa[:, t, c, :],
                                     start=(c == 0), stop=(c == KC - 1))
                z_pss.append(z_ps)
                if t == 0:
                    pre_h1[(0, 0)] = emit_l1(0, 0, 0, w1g0)
            qs, rzps, rzns = [], [], []
            for t in range(T):
                z_ps = z_pss[t]
                az = spool.tile([64, N], f32, tag="az", name=f"az{t}")
                nc.scalar.activation(az[:], z_ps[:], AF.Abs,
                                     bias=rbp[:], scale=1.0)
                e_t = spool.tile([64, N], f32, tag="e", name=f"e{t}")
                nc.scalar.activation(e_t[:], az[:], AF.Exp, scale=-1.0)
                rzp = spool.tile([64, N], f32, tag="rzp", name=f"rzp{t}")
                nc.scalar.activation(rzp[:], z_ps[:], AF.Relu,
                                     bias=rbp[:], scale=1.0)
                rzn = spool.tile([64, N], f32, tag="rzn", name=f"rzn{t}")
                nc.scalar.activation(rzn[:], z_ps[:], AF.Relu,
                                     bias=rbn[:], scale=-1.0)
                qs.append(e_t)
                rzps.append(rzp)
                rzns.append(rzn)
            s_tiles = []
            for t in range(T):
                q_t = qs[t]
                nc.scalar.activation(q_t[:], q_t[:], AF.Ln, bias=1.0,
                                     scale=1.0)
                s_t = spool.tile([128, N], f32r, tag="s", name=f"s{t}")
                nc.vector.scalar_tensor_tensor(
                    s_t[0:64, :], rzns[t][:], -1.0, q_t[:],
                    op0=ALU.mult, op1=ALU.subtract)
                nc.vector.scalar_tensor_tensor(
                    s_t[64:128, :], rzps[t][:], -1.0, q_t[:],
                    op0=ALU.mult, op1=ALU.subtract)
                s_tiles.append(s_t)

            # ---- per-tile output accumulators in SBUF (DVE adds) ----
            out_sb = [opool.tile([OUT, N], f32, tag="o", name=f"out_sb{t}")
                      for t in range(T)]
            pt_tiles = []

            # ---- main loop over 8-leaf groups ----
            def emit_p_block(g, t):
                """A-selection matmul + exp for this group's p values (one
                batch tile); for g=0 also the pT matmul for the b3 term."""
                if g == 0:
                    pt_ps = ps_pm.tile([L, N], f32, tag="pm",
                                       name=f"pt_ps{t}")
                    nc.tensor.matmul(pt_ps[:], a64[:], s_tiles[t][:],
                                     start=True, stop=True)
                    pt_t = ptpool.tile([L, N], f32r, tag="pt", name=f"pt{t}")
                    nc.scalar.activation(pt_t[:], pt_ps[:], AF.Exp, scale=1.0)
                    pt_tiles.append(pt_t)
                pa_ps = ps_pm.tile([128, N], f32, tag="pm",
                                   name=f"pa_ps{g}_{t}")
                nc.tensor.matmul(pa_ps[:], afl[:, 128 * g:128 * (g + 1)],
                                 s_tiles[t][:], start=True, stop=True)
                pa_t = papool.tile([128, N], f32r, tag="pa", name=f"pa{g}_{t}")
                nc.scalar.activation(pa_t[:], pa_ps[:], AF.Exp, scale=1.0)
                return pa_t

            for g in range(NG):
                if g == 0:
                    w1g, w2g, w3g = w1g0, w2g0, w3g0
                else:
                    w2g = h2wpool.tile([128, 4, 2, H2], f32r, tag="w2",
                                       name=f"w2g{g}")
                    nc.sync.dma_start(w2g[:], d_w2[:, 4 * g:4 * (g + 1), :, :])
                    w3g = h2wpool.tile([128, 4, 32], f32r, tag="w3",
                                       name=f"w3g{g}")
                    nc.sync.dma_start(w3g[:], d_w3[:, 4 * g:4 * (g + 1), :])
                    w1g = w1pool.tile([128, 8, KC, 128], f32r, tag="w1",
                                      name=f"w1g{g}")
                    for hc in range(2):
                        nc.sync.dma_start(
                            w1g[:, 4 * hc:4 * (hc + 1), :, :],
                            d_w1[:, 8 * g + 4 * hc:8 * g + 4 * (hc + 1), :, :])
                # For g>0, p values can be computed up front (s_tiles are
                # long done).  For g=0 they are emitted mid-way through the
                # first pair so the PE queue never waits on the routing
                # ACT/DVE chain.
                if g > 0:
                    pa_ts = [emit_p_block(g, t) for t in range(T)]
                else:
                    pa_ts = [None, None]
                prod_ts = [prpool.tile([128, N], f32r, tag="prod",
                                       name=f"prod{g}_{t}") for t in range(T)]
                def emit_l2(s, t, jj, h1_pair):
                    h2pair = h2pool.tile([128, N], f32r, tag="h2s",
                                         name=f"h2p{s}_{t}")
                    for e in range(2):
                        h2_ps = ps_h2.tile([H2, N], f32, tag="h2",
                                           name=f"h2ps{s}_{t}_{e}")
                        nc.tensor.matmul(h2_ps[:], w2g[:, jj, e, :],
                                         h1_pair[e][:],
                                         start=True, stop=True)
                        nc.vector.tensor_scalar(
                            h2pair[64 * e:64 * (e + 1), :], h2_ps[:],
                            b2a[64 * e:64 * (e + 1), s:s + 1], 0.0,
                            op0=ALU.add, op1=ALU.max)
                    return h2pair

                def emit_l3(s, t, jj, h2pair):
                    pred_ps = ps_pm.tile([32, N], f32, tag="pm",
                                         name=f"pred{s}_{t}")
                    nc.tensor.matmul(pred_ps[:], w3g[:, jj, :], h2pair[:],
                                     start=True, stop=True)
                    if g == 0 and jj == 0:
                        pa_ts[t] = emit_p_block(0, t)
                    nc.vector.tensor_mul(
                        prod_ts[t][32 * jj:32 * (jj + 1), :], pred_ps[:],
                        pa_ts[t][32 * jj:32 * (jj + 1), :])

                for jj in range(4):
                    s = 4 * g + jj
                    # pipelined order: L1(t0) -> L1(t1,A) -> L2(t0) ->
                    # L1(t1,B) -> L3(t0) -> L2(t1) -> L3(t1); keeps ACT's
                    # relus and DVE's h2 ops ahead of their PE consumers
                    # while holding at most 3 h1 PSUM banks.
                    if (s, 0) in pre_h1:
                        h1_t0 = pre_h1[(s, 0)]
                    else:
                        h1_t0 = emit_l1(s, 0, jj, w1g)
                    h1_t1 = [emit_l1_leaf(s, 1, jj, 0, w1g)]
                    h2_t0 = emit_l2(s, 0, jj, h1_t0)
                    h1_t1.append(emit_l1_leaf(s, 1, jj, 1, w1g))
                    emit_l3(s, 0, jj, h2_t0)
                    h2_t1 = emit_l2(s, 1, jj, h1_t1)
                    emit_l3(s, 1, jj, h2_t1)
                for t in range(T):
                    if g == 0:
                        rb_ps = ps_pm.tile([OUT, N], f32, tag="pm",
                                           name=f"rb_ps{t}")
                        nc.tensor.matmul(rb_ps[:], b3t[:], pt_tiles[t][:],
                                         start=True, stop=True)
                        nc.vector.tensor_copy(out_sb[t][:], rb_ps[:])
                    r_ps = ps_pm.tile([OUT, N], f32, tag="pm",
                                      name=f"r_ps{g}_{t}")
                    nc.tensor.matmul(r_ps[:], rsel[:], prod_ts[t][:],
                                     start=True, stop=True)
                    nc.vector.tensor_add(out_sb[t][:], out_sb[t][:], r_ps[:])

            # ---- write out ----
            for t in range(T):
                nc.sync.dma_start(d_out[:, N * t:N * (t + 1)], out_sb[t][:])

    try:
        nc.compile()
    finally:
        bacc_mod.get_activation_tables = _orig_get_tables
    return nc


def pack_shared(router_W, router_b, W1, b1, W2, b2, W3, b3):
    """Host-side packing of replicated parameters into SBUF-friendly layouts."""
    f = np.float32
    router_W = np.asarray(router_W, f)
    router_b = np.asarray(router_b, f)
    W1 = np.asarray(W1, f)
    b1 = np.asarray(b1, f)
    W2 = np.asarray(W2, f)
    b2 = np.asarray(b2, f)
    W3 = np.asarray(W3, f)
    b3 = np.asarray(b3, f)

    w1a = np.ascontiguousarray(W1.reshape(L, KC, 128, H1).transpose(2, 0, 1, 3))
    rwa = np.zeros((128, KC, 64), f)
    rwa[:, :, 0:NI] = router_W.T.reshape(KC, 128, NI).transpose(1, 0, 2)
    w2a = np.ascontiguousarray(W2.reshape(NPAIR, 2, H1, H2).transpose(2, 0, 1, 3))

    w3p = np.zeros((NPAIR, 128, 32), f)
    for s in range(NPAIR):
        w3p[s, 0:64, 0:8] = W3[2 * s]
        w3p[s, 64:128, 8:16] = W3[2 * s + 1]
    w3p = np.ascontiguousarray(w3p.transpose(1, 0, 2))

    a64 = np.zeros((128, L), f)
    for leaf in range(L):
        for row in _leaf_path_rows(leaf):
            a64[row, leaf] += 1.0

    afull = np.zeros((128, 1024), f)
    for g in range(NG):
        for jj in range(4):
            s = 4 * g + jj
            for m in range(16):
                leaf = 2 * s + (m >= 8)
                col = 128 * g + 32 * jj + m
                for row in _leaf_path_rows(leaf):
                    afull[row, col] += 1.0

    rsel = np.zeros((128, OUT), f)
    for kk in range(128):
        m = kk % 32
        if m < 8:
            rsel[kk, m] = 1.0
        elif m < 16:
            rsel[kk, m - 8] = 1.0

    return {
        "w1a": w1a,
        "rwa": rwa,
        "w2a": w2a,
        "w3p": w3p,
        "a64": a64,
        "afull": afull,
        "rsel": rsel,
        "b3t": b3,
        "b1a": np.ascontiguousarray(b1.T),
        "b2a": np.ascontiguousarray(b2.reshape(NPAIR, 128).T),
        "rbp": np.concatenate([router_b, [0.0]]).astype(f)[:, None],
        "rbn": np.concatenate([-router_b, [0.0]]).astype(f)[:, None],
    }


def pack_x_core(x_core):
    """[1024, 512] slice -> [128, T, KC, 512] tile-major transposed layout."""
    xc = np.asarray(x_core, np.float32)
    parts = []
    for t in range(T):
        parts.append(xc[N * t:N * (t + 1)].T.reshape(KC, 128, N))
    stacked = np.stack(parts, axis=0)            # [T, KC, 128, N]
    return np.ascontiguousarray(stacked.transpose(2, 0, 1, 3))


_NC_CACHE = {}


def _get_nc():
    if "nc" not in _NC_CACHE:
        _NC_CACHE["nc"] = build_nc()
    return _NC_CACHE["nc"]


def kernel(**inputs):
    x = np.asarray(inputs["x"], np.float32)
    shared = pack_shared(inputs["router_W"], inputs["router_b"],
                         inputs["W1"], inputs["b1"], inputs["W2"],
                         inputs["b2"], inputs["W3"], inputs["b3"])
    in_maps = []
    for i in range(NCORES):
        m = dict(shared)
        m["xa"] = pack_x_core(x[BC * i:BC * (i + 1)])
        in_maps.append(m)
    nc = _get_nc()
    res = run_bass_kernel_spmd(nc, in_maps, core_ids=list(range(NCORES)))
    out = np.concatenate([r["outT"].T for r in res.results], axis=0)
    return np.ascontiguousarray(out, np.float32)



# revision 4
# speedup vs baseline: 1.8474x; 1.8474x over previous
"""Trainium2 Bass kernel for NeuralDecisionTree (soft decision tree MoE).

Strategy: data-parallel over batch across 8 NeuronCores (1024 rows/core),
weights replicated.  All matmuls run in float32r (reduced-precision fp32
multiply, fp32 accumulate) at full PE streaming rate.

Per-core dataflow (activations kept in [feature, batch] layout throughout):
  router:  z = router_W @ x^T          (4 K-chunk matmuls, M padded to 64)
  S      = [ln s; 0; ln(1-s); 0]       (128 rows; s = sigmoid(z + rb))
  log p  = A @ S  -> p = exp(A @ S)    per-leaf soft routing products in
           log space via 0/1 selection matmuls; pre-broadcast into the
           pred layout (8 output slots per leaf).
  L1:     h1T_l = relu(W1_l^T @ x^T + b1)      per leaf, 4 K-chunk matmuls
  L2:     two M-stacked matmuls accumulate the leaf pair into one
          [128,N] PSUM bank ([W2A|0] then [0|W2B]); one bias+relu op.
  L3:     pred for a pair via [128,32] pack -> [32,N] PSUM written at
          partition offset 32*jj, packing 4 pairs into one bank.
  mix:    prod = (pred + b3bc) * pa in one STT op per group-tile,
          out_t += R^T @ prod (0/1 matmul); b3 handled inside the STT.
"""

import os
import sys

import numpy as np

if "/opt/trn_rl_repo" not in sys.path:
    sys.path.insert(0, "/opt/trn_rl_repo")

import concourse.bass as bass
import concourse.hw_specs as hw_specs
import concourse.tile as tile
from concourse import bacc, mybir
from concourse.bass_utils import run_bass_kernel_spmd

_ONE_TABLE = "natural_log_exp_and_others"
_orig_get_tables = hw_specs.get_activation_tables


def _patched_get_tables(module_arch):
    """Confine activation-table choice to one set that covers every ACT
    func this kernel uses (exp/ln/relu/abs/copy/identity), so the greedy
    per-instruction table picker never ping-pongs between sets.  Dict
    order (= act_func_set_id) is preserved; other sets are emptied."""
    tables = dict(_orig_get_tables(module_arch))
    keep = tables[_ONE_TABLE]
    return {k: (v if k == _ONE_TABLE else (v & set()) or set())
            if k != _ONE_TABLE else keep for k, v in tables.items()}

f32 = mybir.dt.float32
f32r = mybir.dt.float32r
AF = mybir.ActivationFunctionType
ALU = mybir.AluOpType

# Problem shape (hardcoded; harness contract)
B = 8192
D = 512
H1 = 128
H2 = 64
OUT = 8
L = 64
NI = 63
NCORES = 8
BC = B // NCORES        # 1024 rows per core
N = 512                 # batch tile (matmul free dim / PSUM bank)
T = BC // N             # 2 batch tiles per core
KC = D // 128           # 4 contraction chunks for the input dim
NPAIR = L // 2          # 32 leaf pairs
NG = 8                  # 8-leaf groups


def _leaf_path_rows(leaf):
    """Rows of the [128] log-sigmoid stack contributing to log p(leaf).

    Row n (n<63) holds ln d_n; row 64+n holds ln(1-d_n); rows 63 and 127
    are zero pads.  Mirrors the reference's level-wise p interleave.
    """
    rows = []
    for k in range(6):
        prefix = leaf >> (6 - k)
        node = (2 ** k - 1) + prefix
        bit = (leaf >> (5 - k)) & 1
        rows.append(node + 64 * bit)
    return rows


def build_nc():
    nc = bacc.Bacc("TRN2", target_bir_lowering=False, debug=False,
                   num_devices=NCORES)
    bacc_mod = sys.modules["concourse.bacc"]
    bacc_mod.get_activation_tables = _patched_get_tables

    d_xa = nc.dram_tensor("xa", [128, T, KC, N], f32r, kind="ExternalInput").ap()
    d_w1 = nc.dram_tensor("w1a", [128, L, KC, 128], f32r, kind="ExternalInput").ap()
    d_rw = nc.dram_tensor("rwa", [128, KC, 64], f32r, kind="ExternalInput").ap()
    d_w2 = nc.dram_tensor("w2p", [128, NPAIR, 2, 128], f32r, kind="ExternalInput").ap()
    d_w3 = nc.dram_tensor("w3p", [128, NPAIR, 32], f32r, kind="ExternalInput").ap()
    d_afl = nc.dram_tensor("afull", [128, 1024], f32r, kind="ExternalInput").ap()
    d_r = nc.dram_tensor("rsel", [128, OUT], f32r, kind="ExternalInput").ap()
    d_b3bc = nc.dram_tensor("b3bc", [128, NG], f32, kind="ExternalInput").ap()
    d_b1 = nc.dram_tensor("b1a", [128, L], f32, kind="ExternalInput").ap()
    d_b2 = nc.dram_tensor("b2a", [128, NPAIR], f32, kind="ExternalInput").ap()
    d_rbp = nc.dram_tensor("rbp", [64, 1], f32, kind="ExternalInput").ap()
    d_rbn = nc.dram_tensor("rbn", [64, 1], f32, kind="ExternalInput").ap()
    d_out = nc.dram_tensor("outT", [OUT, BC], f32, kind="ExternalOutput").ap()

    with tile.TileContext(nc) as tc:
        with tc.tile_pool(name="const", bufs=1) as cpool, \
             tc.tile_pool(name="w1", bufs=2) as w1pool, \
             tc.tile_pool(name="w2w3", bufs=3) as h2wpool, \
             tc.tile_pool(name="spool", bufs=2) as spool, \
             tc.tile_pool(name="parr", bufs=3) as papool, \
             tc.tile_pool(name="h1", bufs=4) as h1pool, \
             tc.tile_pool(name="h2", bufs=3) as h2pool, \
             tc.tile_pool(name="prod", bufs=3) as prpool, \
             tc.tile_pool(name="osb", bufs=2) as opool, \
             tc.tile_pool(name="ps_h1", bufs=3, space="PSUM") as ps_h1, \
             tc.tile_pool(name="ps_h2", bufs=2, space="PSUM") as ps_h2, \
             tc.tile_pool(name="ps_pm", bufs=3, space="PSUM") as ps_pm:
            # banks: 3 (h1) + 2 (h2 pair-stacked) + 3 (z/pa/pred/r) = 8

            # ---- constants into SBUF, in byte-arrival order ----
            # One serial sync queue controls exactly which bytes land first:
            # router weights (warmup/z) -> x -> g0's L2/L3 weights + A
            # matrices -> W1 pair chunks.  Tiny biases ride the scalar queue.
            rwa = cpool.tile([128, KC, 64], f32r)
            nc.sync.dma_start(rwa[:], d_rw)
            xa = cpool.tile([128, T, KC, N], f32r)
            for c in range(KC):
                nc.sync.dma_start(xa[:, 0, c, :], d_xa[:, 0, c, :])
            w1g0 = w1pool.tile([128, 8, KC, 128], f32r, tag="w1", name="w1g0")
            nc.sync.dma_start(w1g0[:, 0:2, :, :], d_w1[:, 0:2, :, :])
            w2g0 = h2wpool.tile([128, 4, 2, 128], f32r, tag="w2", name="w2g0")
            nc.sync.dma_start(w2g0[:], d_w2[:, 0:4, :, :])
            w3g0 = h2wpool.tile([128, 4, 32], f32r, tag="w3", name="w3g0")
            nc.sync.dma_start(w3g0[:], d_w3[:, 0:4, :])
            afl = cpool.tile([128, 1024], f32r)
            nc.sync.dma_start(afl[:, 0:256], d_afl[:, 0:256])
            for c in range(KC):
                nc.sync.dma_start(xa[:, 1, c, :], d_xa[:, 1, c, :])
            nc.sync.dma_start(w1g0[:, 2:4, :, :], d_w1[:, 2:4, :, :])
            nc.sync.dma_start(afl[:, 256:1024], d_afl[:, 256:1024])
            rsel = cpool.tile([128, OUT], f32r)
            nc.sync.dma_start(rsel[:], d_r)
            nc.sync.dma_start(w1g0[:, 4:6, :, :], d_w1[:, 4:6, :, :])
            nc.sync.dma_start(w1g0[:, 6:8, :, :], d_w1[:, 6:8, :, :])
            rbp = cpool.tile([64, 1], f32)
            nc.scalar.dma_start(rbp[:], d_rbp)
            rbn = cpool.tile([64, 1], f32)
            nc.scalar.dma_start(rbn[:], d_rbn)
            b1a = cpool.tile([128, L], f32)
            nc.scalar.dma_start(b1a[:], d_b1)
            b2a = cpool.tile([128, NPAIR], f32)
            nc.scalar.dma_start(b2a[:], d_b2)
            b3bc = cpool.tile([128, NG], f32)
            nc.scalar.dma_start(b3bc[:], d_b3bc)

            # ---- routing + first L1 pair, interleaved for startup ----
            # PE order: z(t0) -> L1 pair0 t0 (needs only the first W1 chunk)
            # -> z(t1) (by which time the second half of x has landed).
            # Row 63 of z is a zero pad from the padded router weights.
            #   ln s     = -(relu(-z') + ln(1 + exp(-|z'|)))
            #   ln (1-s) = -(relu( z') + ln(1 + exp(-|z'|)))
            def emit_l1_leaf(s, t, jj, e, w1g):
                leaf = 2 * s + e
                h1_ps = ps_h1.tile([128, N], f32, tag="h1",
                                   name=f"h1ps{s}_{t}_{e}")
                for c in range(KC):
                    nc.tensor.matmul(
                        h1_ps[:], w1g[:, 2 * jj + e, c, :],
                        xa[:, t, c, :],
                        start=(c == 0), stop=(c == KC - 1))
                h1_t = h1pool.tile([128, N], f32r, tag="h1s",
                                   name=f"h1s{s}_{t}_{e}")
                nc.scalar.activation(h1_t[:], h1_ps[:], AF.Relu,
                                     bias=b1a[:, leaf:leaf + 1],
                                     scale=1.0)
                return h1_t

            def emit_l1(s, t, jj, w1g):
                return [emit_l1_leaf(s, t, jj, e, w1g) for e in range(2)]

            z_pss = []
            pre_h1 = {}
            for t in range(T):
                z_ps = ps_pm.tile([64, N], f32, tag="pm", name=f"z_ps{t}")
                for c in range(KC):
                    nc.tensor.matmul(z_ps[:], rwa[:, c, :],
                                     xa[:, t, c, :],
                                     start=(c == 0), stop=(c == KC - 1))
                z_pss.append(z_ps)
                if t == 0:
                    pre_h1[(0, 0)] = emit_l1(0, 0, 0, w1g0)
            qs, rzps, rzns = [], [], []
            for t in range(T):
                z_ps = z_pss[t]
                az = spool.tile([64, N], f32, tag="az", name=f"az{t}")
                nc.scalar.activation(az[:], z_ps[:], AF.Abs,
                                     bias=rbp[:], scale=1.0)
                e_t = spool.tile([64, N], f32, tag="e", name=f"e{t}")
                nc.scalar.activation(e_t[:], az[:], AF.Exp, scale=-1.0)
                rzp = spool.tile([64, N], f32, tag="rzp", name=f"rzp{t}")
                nc.scalar.activation(rzp[:], z_ps[:], AF.Relu,
                                     bias=rbp[:], scale=1.0)
                rzn = spool.tile([64, N], f32, tag="rzn", name=f"rzn{t}")
                nc.scalar.activation(rzn[:], z_ps[:], AF.Relu,
                                     bias=rbn[:], scale=-1.0)
                qs.append(e_t)
                rzps.append(rzp)
                rzns.append(rzn)
            s_tiles = []
            for t in range(T):
                q_t = qs[t]
                nc.scalar.activation(q_t[:], q_t[:], AF.Ln, bias=1.0,
                                     scale=1.0)
                s_t = spool.tile([128, N], f32r, tag="s", name=f"s{t}")
                nc.vector.scalar_tensor_tensor(
                    s_t[0:64, :], rzns[t][:], -1.0, q_t[:],
                    op0=ALU.mult, op1=ALU.subtract)
                nc.vector.scalar_tensor_tensor(
                    s_t[64:128, :], rzps[t][:], -1.0, q_t[:],
                    op0=ALU.mult, op1=ALU.subtract)
                s_tiles.append(s_t)

            # ---- per-tile output accumulators in SBUF (DVE adds) ----
            out_sb = [opool.tile([OUT, N], f32, tag="o", name=f"out_sb{t}")
                      for t in range(T)]

            # ---- main loop over 8-leaf groups ----
            def emit_p_block(g, t):
                """A-selection matmul + exp for this group's p values (one
                batch tile), pre-broadcast into the pred row layout."""
                pa_ps = ps_pm.tile([128, N], f32, tag="pm",
                                   name=f"pa_ps{g}_{t}")
                nc.tensor.matmul(pa_ps[:], afl[:, 128 * g:128 * (g + 1)],
                                 s_tiles[t][:], start=True, stop=True)
                pa_t = papool.tile([128, N], f32r, tag="pa", name=f"pa{g}_{t}")
                nc.scalar.activation(pa_t[:], pa_ps[:], AF.Exp, scale=1.0)
                return pa_t

            for g in range(NG):
                if g == 0:
                    w1g, w2g, w3g = w1g0, w2g0, w3g0
                else:
                    w2g = h2wpool.tile([128, 4, 2, 128], f32r, tag="w2",
                                       name=f"w2g{g}")
                    nc.sync.dma_start(w2g[:], d_w2[:, 4 * g:4 * (g + 1), :, :])
                    w3g = h2wpool.tile([128, 4, 32], f32r, tag="w3",
                                       name=f"w3g{g}")
                    nc.sync.dma_start(w3g[:], d_w3[:, 4 * g:4 * (g + 1), :])
                    w1g = w1pool.tile([128, 8, KC, 128], f32r, tag="w1",
                                      name=f"w1g{g}")
                    for hc in range(2):
                        nc.sync.dma_start(
                            w1g[:, 4 * hc:4 * (hc + 1), :, :],
                            d_w1[:, 8 * g + 4 * hc:8 * g + 4 * (hc + 1), :, :])
                # p-broadcast values for this group.  For g>0 emit them up
                # front (s_tiles are long done); for g=0 they are emitted
                # mid-way through the first pair so the PE queue never waits
                # on the routing ACT/DVE chain.
                if g > 0:
                    pa_ts = [emit_p_block(g, t) for t in range(T)]
                else:
                    pa_ts = [None, None]
                prod_ts = [prpool.tile([128, N], f32r, tag="prod",
                                       name=f"prod{g}_{t}") for t in range(T)]

                def emit_l2(s, t, jj, h1_pair):
                    """Two M-stacked matmuls accumulate the pair into one
                    [128,N] PSUM bank; single bias+relu (DVE) drains it."""
                    h2_ps = ps_h2.tile([128, N], f32, tag="h2",
                                       name=f"h2ps{s}_{t}")
                    nc.tensor.matmul(h2_ps[:], w2g[:, jj, 0, :],
                                     h1_pair[0][:], start=True, stop=False)
                    nc.tensor.matmul(h2_ps[:], w2g[:, jj, 1, :],
                                     h1_pair[1][:], start=False, stop=True)
                    h2pair = h2pool.tile([128, N], f32r, tag="h2s",
                                         name=f"h2p{s}_{t}")
                    nc.vector.tensor_scalar(
                        h2pair[:], h2_ps[:], b2a[:, s:s + 1], 0.0,
                        op0=ALU.add, op1=ALU.max)
                    return h2pair

                def emit_l3(s, t, jj, h2pair):
                    """Pair pred into [32,N] PSUM; prod row block gets
                    (pred + b3) * p in a single STT op (b3 folded here)."""
                    pred_ps = ps_pm.tile([32, N], f32, tag="pm",
                                         name=f"pred{s}_{t}")
                    nc.tensor.matmul(pred_ps[:], w3g[:, jj, :], h2pair[:],
                                     start=True, stop=True)
                    if g == 0 and jj == 0:
                        pa_ts[t] = emit_p_block(0, t)
                    nc.vector.scalar_tensor_tensor(
                        prod_ts[t][32 * jj:32 * (jj + 1), :], pred_ps[:],
                        b3bc[32 * jj:32 * (jj + 1), g:g + 1],
                        pa_ts[t][32 * jj:32 * (jj + 1), :],
                        op0=ALU.add, op1=ALU.mult)

                for jj in range(4):
                    s = 4 * g + jj
                    # pipelined order: L1(t0) -> L1(t1,A) -> L2(t0) ->
                    # L1(t1,B) -> L3(t0) -> L2(t1) -> L3(t1); keeps ACT's
                    # relus and DVE's h2 ops ahead of their PE consumers
                    # while holding at most 3 h1 PSUM banks.
                    if (s, 0) in pre_h1:
                        h1_t0 = pre_h1[(s, 0)]
                    else:
                        h1_t0 = emit_l1(s, 0, jj, w1g)
                    h1_t1 = [emit_l1_leaf(s, 1, jj, 0, w1g)]
                    h2_t0 = emit_l2(s, 0, jj, h1_t0)
                    h1_t1.append(emit_l1_leaf(s, 1, jj, 1, w1g))
                    emit_l3(s, 0, jj, h2_t0)
                    h2_t1 = emit_l2(s, 1, jj, h1_t1)
                    emit_l3(s, 1, jj, h2_t1)
                # ---- mix reduction for this group ----
                for t in range(T):
                    r_ps = ps_pm.tile([OUT, N], f32, tag="pm",
                                      name=f"r_ps{g}_{t}")
                    nc.tensor.matmul(r_ps[:], rsel[:], prod_ts[t][:],
                                     start=True, stop=True)
                    if g == 0:
                        nc.vector.tensor_copy(out_sb[t][:], r_ps[:])
                    else:
                        nc.vector.tensor_add(out_sb[t][:], out_sb[t][:],
                                             r_ps[:])
                    if g == NG - 1:
                        nc.sync.dma_start(d_out[:, N * t:N * (t + 1)],
                                          out_sb[t][:])

    try:
        nc.compile()
    finally:
        bacc_mod.get_activation_tables = _orig_get_tables
    return nc


def pack_shared(router_W, router_b, W1, b1, W2, b2, W3, b3):
    """Host-side packing of replicated parameters into SBUF-friendly layouts."""
    f = np.float32
    router_W = np.asarray(router_W, f)
    router_b = np.asarray(router_b, f)
    W1 = np.asarray(W1, f)
    b1 = np.asarray(b1, f)
    W2 = np.asarray(W2, f)
    b2 = np.asarray(b2, f)
    W3 = np.asarray(W3, f)
    b3 = np.asarray(b3, f)

    w1a = np.ascontiguousarray(W1.reshape(L, KC, 128, H1).transpose(2, 0, 1, 3))
    rwa = np.zeros((128, KC, 64), f)
    rwa[:, :, 0:NI] = router_W.T.reshape(KC, 128, NI).transpose(1, 0, 2)

    w2p = np.zeros((NPAIR, 128, 2, 128), f)
    for s in range(NPAIR):
        w2p[s, :, 0, 0:64] = W2[2 * s]
        w2p[s, :, 1, 64:128] = W2[2 * s + 1]
    w2p = np.ascontiguousarray(w2p.transpose(1, 0, 2, 3))

    w3p = np.zeros((NPAIR, 128, 32), f)
    for s in range(NPAIR):
        w3p[s, 0:64, 0:8] = W3[2 * s]
        w3p[s, 64:128, 8:16] = W3[2 * s + 1]
    w3p = np.ascontiguousarray(w3p.transpose(1, 0, 2))

    afull = np.zeros((128, 1024), f)
    for g in range(NG):
        for jj in range(4):
            s = 4 * g + jj
            for m in range(16):
                leaf = 2 * s + (m >= 8)
                col = 128 * g + 32 * jj + m
                for row in _leaf_path_rows(leaf):
                    afull[row, col] += 1.0

    rsel = np.zeros((128, OUT), f)
    for kk in range(128):
        m = kk % 32
        if m < 8:
            rsel[kk, m] = 1.0
        elif m < 16:
            rsel[kk, m - 8] = 1.0

    b3bc = np.zeros((128, NG), f)
    for g in range(NG):
        for jj in range(4):
            s = 4 * g + jj
            for m in range(16):
                leaf = 2 * s + (m >= 8)
                b3bc[32 * jj + m, g] = b3[leaf, m % 8]

    return {
        "w1a": w1a,
        "rwa": rwa,
        "w2p": w2p,
        "w3p": w3p,
        "afull": afull,
        "rsel": rsel,
        "b3bc": b3bc,
        "b1a": np.ascontiguousarray(b1.T),
        "b2a": np.ascontiguousarray(b2.reshape(NPAIR, 128).T),
        "rbp": np.concatenate([router_b, [0.0]]).astype(f)[:, None],
        "rbn": np.concatenate([-router_b, [0.0]]).astype(f)[:, None],
    }


def pack_x_core(x_core):
    """[1024, 512] slice -> [128, T, KC, 512] tile-major transposed layout."""
    xc = np.asarray(x_core, np.float32)
    parts = []
    for t in range(T):
        parts.append(xc[N * t:N * (t + 1)].T.reshape(KC, 128, N))
    stacked = np.stack(parts, axis=0)            # [T, KC, 128, N]
    return np.ascontiguousarray(stacked.transpose(2, 0, 1, 3))


_NC_CACHE = {}


def _get_nc():
    if "nc" not in _NC_CACHE:
        _NC_CACHE["nc"] = build_nc()
    return _NC_CACHE["nc"]


def kernel(**inputs):
    x = np.asarray(inputs["x"], np.float32)
    shared = pack_shared(inputs["router_W"], inputs["router_b"],
                         inputs["W1"], inputs["b1"], inputs["W2"],
                         inputs["b2"], inputs["W3"], inputs["b3"])
    in_maps = []
    for i in range(NCORES):
        m = dict(shared)
        m["xa"] = pack_x_core(x[BC * i:BC * (i + 1)])
        in_maps.append(m)
    nc = _get_nc()
    res = run_bass_kernel_spmd(nc, in_maps, core_ids=list(range(NCORES)))
    out = np.concatenate([r["outT"].T for r in res.results], axis=0)
    return np.ascontiguousarray(out, np.float32)
